# revision 26
# baseline (speedup 1.0000x reference)
"""Trainium2 Bass kernel for location-sensitive attention (Tacotron-style).

Computes, for B=256, T=1024, RNN_DIM=1024, EMB_DIM=512, ATT_DIM=128:
  pq   = query @ w_query.T                      (B, 128)
  conv = conv1d(attention_weights_cat, w)       (B, 32, T)  'same' pad
  ploc = conv.T @ w_loc_lin.T                   (B, T, 128)
  e    = tanh(pq + ploc + processed_memory) @ v (B, T)
  w    = softmax(e + mask_bias, axis=T)         (B, T)
  ctx  = w @ memory                             (B, 512)
returns (ctx, w).

Sharding: pure data parallel over batch, 32 rows per core on 8 cores.

Design notes (measured on trn2, NEFF exec ~414us vs ~675us for the first
correct fp32 version):
  - fp32 matmuls on trn2 run 2-pass (LOW_HIGH) at ~1/4 bf16 streaming rate,
    so every bandwidth-relevant matmul runs in fp16 with f32 PSUM
    accumulation (end-to-end rel err ~3.5e-4; set CTX_DT = F32 to trade
    ~180us for ~1e-6 accuracy).
  - conv+linear are folded on the host into one (62, 128) matrix W2; the
    conv becomes 2 matmuls per row over an im2col built by
    overlapping-window DMA reads of the host-padded attention weights,
    batched 4 rows per DMA.  Output lands in [a, t] layout.
  - processed_memory is transposed to [a, t] and cast to fp16 on the host;
    VectorE adds it to the conv PSUM; ScalarE applies tanh with the pq
    per-partition bias fused in.
  - energies = v^T @ tanh(...) (M=1 matvecs) are staged through partition 0
    and spread into per-half [16, T] softmax tiles by SBUF->SBUF DMAs;
    softmax is batched across partitions; weights are PE-transposed into
    [t, row] fp16 columns for the context step.
  - context: per row, 8 accumulating [128_t,1]x[128_t,512] fp16 matvecs
    over memory tiles cast f32->fp16 during the (SWDGE) DMA; the second
    half's energies overlap the first half's context to keep HBM busy.
  - phase-1 work is software-pipelined in groups (front: conv/add/tanh;
    back: v-reduce/stage) to hide cross-engine semaphore latency.
"""
import numpy as np

import concourse.bass as bass
import concourse.bacc as bacc
import concourse.mybir as mybir
import concourse.tile as tile
from concourse.bass_utils import run_bass_kernel_spmd

F32 = mybir.dt.float32
# context-path dtype: fp16 keeps ctx relerr ~3e-4 (bf16: 2.5e-3, f32 exact
# but 2x matmul passes).
CTX_DT = mybir.dt.float16

B, T = 256, 1024
RNN_DIM, EMB_DIM, ATT_DIM = 1024, 512, 128
N_FILT, KSIZE = 32, 31
PAD = (KSIZE - 1) // 2
NCORES = 8
BL = B // NCORES          # 32 batch rows per core
CK = 2 * KSIZE            # 62 im2col rows
PADT = T + 2 * PAD        # 1054
NTCH = T // 128           # 8 t-chunks of 128


def build_nc():
    nc = bacc.Bacc("TRN2", target_bir_lowering=False)

    # ---- per-core inputs ----
    awp = nc.declare_dram_parameter("awp", [BL, 2, PADT], CTX_DT, isOutput=False)
    qT = nc.declare_dram_parameter("qT", [RNN_DIM, BL], CTX_DT, isOutput=False)
    pmT = nc.declare_dram_parameter("pmT", [BL, ATT_DIM, T], CTX_DT, isOutput=False)
    mem = nc.declare_dram_parameter("mem", [BL, T, EMB_DIM], F32, isOutput=False)
    mneg = nc.declare_dram_parameter("mneg", [BL, T], F32, isOutput=False)
    # ---- shared params ----
    wqT = nc.declare_dram_parameter("wqT", [RNN_DIM, ATT_DIM], CTX_DT, isOutput=False)
    w2 = nc.declare_dram_parameter("w2", [CK, ATT_DIM], CTX_DT, isOutput=False)
    vcol = nc.declare_dram_parameter("vcol", [ATT_DIM, 1], CTX_DT, isOutput=False)
    ident = nc.declare_dram_parameter("ident", [128, 128], F32, isOutput=False)
    # ---- outputs ----
    ctx_out = nc.declare_dram_parameter("ctx_out", [BL, EMB_DIM], F32, isOutput=True)
    attw_out = nc.declare_dram_parameter("attw_out", [BL, T], F32, isOutput=True)

    with tile.TileContext(nc) as tc:
        with (
            tc.tile_pool(name="const", bufs=1) as cp,
            tc.tile_pool(name="xp", bufs=4) as xp,
            tc.tile_pool(name="pmp", bufs=4) as pmp,
            tc.tile_pool(name="sp", bufs=4) as sp,
            tc.tile_pool(name="memp", bufs=10) as memp,
            tc.tile_pool(name="stg", bufs=2) as stg,
            tc.tile_pool(name="pta", bufs=2, space="PSUM") as pta,
            tc.tile_pool(name="psB", bufs=4, space="PSUM") as psB,
        ):
            # ---------- constants ----------
            wqT_t = cp.tile([128, RNN_DIM // 128, ATT_DIM], CTX_DT, tag="wqT")
            nc.sync.dma_start(
                wqT_t[:], wqT[:].rearrange("(ih il) a -> il ih a", il=128))
            qT_t = cp.tile([128, RNN_DIM // 128, BL], CTX_DT, tag="qT")
            nc.sync.dma_start(
                qT_t[:], qT[:].rearrange("(ih il) b -> il ih b", il=128))
            w2_t = cp.tile([CK, ATT_DIM], CTX_DT, tag="w2")
            nc.sync.dma_start(w2_t[:], w2[:])
            v_t = cp.tile([ATT_DIM, 1], CTX_DT, tag="v")
            nc.sync.dma_start(v_t[:], vcol[:])
            id_t = cp.tile([128, 128], F32, tag="id")
            nc.sync.dma_start(id_t[:], ident[:])
            mneg_half = []
            for x in range(2):
                mh = cp.tile([BL // 2, T], F32, tag=f"mneg{x}", name=f"mneg{x}")
                nc.sync.dma_start(mh[:], mneg[x * (BL // 2):(x + 1) * (BL // 2)])
                mneg_half.append(mh)

            pq_sb = cp.tile([ATT_DIM, BL], F32, tag="pq")
            HB = BL // 2  # half-batch: softmax/context run per 16-row half
            e_half = [cp.tile([HB, T], F32, tag=f"e{x}", name=f"e_sb{x}")
                      for x in range(2)]
            w_half = [cp.tile([HB, T], F32, tag=f"w{x}", name=f"w_sb{x}")
                      for x in range(2)]
            wT_half = [cp.tile([128, NTCH, HB], CTX_DT, tag=f"wT{x}",
                               name=f"wT_sb{x}") for x in range(2)]
            stat_half = [cp.tile([HB, 4], F32, tag=f"st{x}", name=f"stat{x}")
                         for x in range(2)]

            # ---------- pq = w_query @ query^T : [128_a, BL] ----------
            pqp = pta.tile([ATT_DIM, BL], F32, tag="pta", name="pqp")
            for c in range(RNN_DIM // 128):
                nc.tensor.matmul(
                    pqp[:], wqT_t[:, c, :], qT_t[:, c, :],
                    start=(c == 0), stop=(c == RNN_DIM // 128 - 1))
            nc.vector.tensor_copy(pq_sb[:], pqp[:])

            # ---------- mem streaming (cast to CTX_DT during DMA) ----------
            mem_tiles = {}

            def load_mem(b, h):
                mt = memp.tile([128, 4, 512], CTX_DT, tag="mt", name=f"mt_{b}_{h}")
                nc.gpsimd.dma_start(
                    mt[:],
                    mem[b, h * 512:(h + 1) * 512, :].rearrange(
                        "(th tl) d -> tl th d", tl=128))
                mem_tiles[(b, h)] = mt

            prefetch = [(b, h) for b in range(BL) for h in range(2)][:12]

            # ---------- phase 1: energies ----------
            # Software-pipelined by groups of GRP rows: the PE stream is
            # [convs(g), vreds(g-1)] so cross-engine latency (PE->DVE add ->
            # ACT tanh -> PE vred) is hidden a group at a time instead of
            # stalling every row.
            GRP = 4
            LOOKAHEAD = 12
            QB = 4   # rows per input-DMA batch
            th_tiles = {}
            ev_tiles = {}
            estages = {}
            in_tiles = {}

            def emit_dma(b0):
                # one batched im2col DMA pair + one pm DMA covering QB rows
                if b0 >= BL:
                    return
                xt = xp.tile([CK, QB, T], CTX_DT, tag="x", name=f"x_{b0}")
                for c2 in range(2):
                    nc.sync.dma_start(
                        xt[c2 * KSIZE:(c2 + 1) * KSIZE, :, :],
                        bass.AP(awp, (b0 * 2 + c2) * PADT,
                                [[1, KSIZE], [2 * PADT, QB], [1, T]]))
                pmt = pmp.tile([128, QB, T], CTX_DT, tag="pm", name=f"pm_{b0}")
                nc.sync.dma_start(
                    pmt[:], pmT[b0:b0 + QB].rearrange("b a t -> a b t"))
                for b in range(b0, b0 + QB):
                    in_tiles[b] = (xt, pmt)

            for b0 in range(0, LOOKAHEAD, QB):
                emit_dma(b0)

            def emit_front(b):
                xt4, pmt4 = in_tiles.pop(b)
                if b % QB == 0:
                    emit_dma(b + LOOKAHEAD)
                xt = xt4[:, b % QB, :]
                pmt = pmt4[:, b % QB, :]
                pl = pta.tile([128, T], F32, tag="pta", name=f"pl_{b}")
                for h in range(2):
                    nc.tensor.matmul(
                        pl[:, h * 512:(h + 1) * 512], w2_t[:],
                        xt[:, h * 512:(h + 1) * 512],
                        start=True, stop=True)
                s = sp.tile([128, T], F32, tag="s", bufs=4, name=f"s_{b}")
                nc.vector.tensor_add(s[:], pl[:], pmt[:])
                th = sp.tile([128, T], CTX_DT, tag="ths", bufs=8,
                             name=f"th_{b}")
                nc.scalar.activation(
                    th[:], s[:], mybir.ActivationFunctionType.Tanh,
                    bias=pq_sb[:, b:b + 1], scale=1.0)
                th_tiles[b] = th

            def emit_back(b):
                if b % 2 == 0:
                    estages[b // 2] = stg.tile([1, 2 * T], F32, tag="est",
                                               bufs=3, name=f"est_{b}")
                estage = estages[b // 2]
                th = th_tiles.pop(b)
                for h in range(2):
                    ev = psB.tile([1, 512], F32, tag="vec", name=f"ev_{b}_{h}")
                    nc.tensor.matmul(ev[:], v_t[:],
                                     th[:, h * 512:(h + 1) * 512],
                                     start=True, stop=True)
                    ev_tiles[(b, h)] = (ev, estage)

            def drain_back(b):
                for h in range(2):
                    ev, estage = ev_tiles.pop((b, h))
                    nc.vector.tensor_copy(
                        estage[0:1,
                               (b % 2) * T + h * 512:(b % 2) * T + (h + 1) * 512],
                        ev[:])
                if b % 2 == 1:
                    eh = e_half[b // HB]
                    r = b % HB
                    nc.gpsimd.dma_start(eh[r - 1:r + 1, :],
                                        estages.pop(b // 2)[:])

            def softmax_half(x):
                eh, wh, st, mh = e_half[x], w_half[x], stat_half[x], mneg_half[x]
                nc.vector.tensor_add(eh[:], eh[:], mh[:])
                nc.vector.reduce_max(st[:, 0:1], eh[:], axis=mybir.AxisListType.X)
                nc.vector.tensor_scalar_mul(st[:, 1:2], st[:, 0:1], -1.0)
                nc.scalar.activation(
                    wh[:], eh[:], mybir.ActivationFunctionType.Exp,
                    bias=st[:, 1:2], scale=1.0, accum_out=st[:, 2:3])
                nc.vector.reciprocal(st[:, 3:4], st[:, 2:3])
                nc.vector.tensor_scalar_mul(wh[:], wh[:], st[:, 3:4])
                nc.gpsimd.dma_start(attw_out[x * HB:(x + 1) * HB, :], wh[:])
                for c in range(NTCH):
                    wTp = pta.tile([128, HB], F32, tag="pta", name=f"wTp{x}_{c}")
                    nc.tensor.matmul(
                        wTp[:], wh[:, c * 128:(c + 1) * 128], id_t[0:HB, 0:HB],
                        is_transpose=True, start=True, stop=True)
                    nc.vector.tensor_copy(wT_half[x][:, c, :], wTp[:])

            cstages = {}

            def emit_ctx(b):
                for h in range(2):
                    if (b, h) not in mem_tiles:
                        load_mem(b, h)
                if b % 4 == 0:
                    cstages[b // 4] = stg.tile([1, 4 * EMB_DIM], F32, tag="cst",
                                               name=f"cst_{b}")
                cstage = cstages[b // 4]
                cx = psB.tile([1, EMB_DIM], F32, tag="vec", name=f"cx_{b}")
                wTh = wT_half[b // HB]
                r = b % HB
                for h in range(2):
                    mt = mem_tiles.pop((b, h))
                    for j in range(4):
                        c = h * 4 + j
                        nc.tensor.matmul(
                            cx[:], wTh[:, c, r:r + 1], mt[:, j, :],
                            start=(c == 0), stop=(c == 7))
                nc.scalar.copy(
                    cstage[0:1, (b % 4) * EMB_DIM:(b % 4 + 1) * EMB_DIM], cx[:])
                if b % 4 == 3:
                    nc.gpsimd.dma_start(ctx_out[b - 3:b + 1, :],
                                        cstages.pop(b // 4)[:])

            # half A energies
            for b in range(HB):
                emit_front(b)
                if b - GRP >= 0:
                    emit_back(b - GRP)
                    drain_back(b - GRP)
                if b >= 6 and b % 2 == 1 and prefetch:
                    for _ in range(3):
                        if prefetch:
                            load_mem(*prefetch.pop(0))
            for b in range(HB - GRP, HB):
                emit_back(b)
                drain_back(b)
            softmax_half(0)
            # half B energies interleaved with half A context
            for i in range(HB):
                b = HB + i
                emit_front(b)
                if b - GRP >= HB:
                    emit_back(b - GRP)
                    drain_back(b - GRP)
                emit_ctx(i)
            for b in range(BL - GRP, BL):
                emit_back(b)
                drain_back(b)
            softmax_half(1)
            for i in range(HB, BL):
                emit_ctx(i)

    nc.compile()
    return nc


_NC = None


def get_nc():
    global _NC
    if _NC is None:
        _NC = build_nc()
    return _NC


def host_prep(query, memory, processed_memory, attention_weights_cat, mask,
              w_query, w_loc_conv, w_loc_lin, v):
    """Build the per-core input maps (host-side sharding + weight folding)."""
    query = np.asarray(query, np.float32)
    memory = np.ascontiguousarray(np.asarray(memory, np.float32))
    pm_np = np.asarray(processed_memory, np.float32)
    aw = np.asarray(attention_weights_cat, np.float32)
    mask = np.asarray(mask)
    w_query = np.asarray(w_query, np.float32)
    w_loc_conv = np.asarray(w_loc_conv, np.float32)
    w_loc_lin = np.asarray(w_loc_lin, np.float32)
    v = np.asarray(v, np.float32)

    awp = np.zeros((B, 2, PADT), np.float16)
    awp[:, :, PAD:PAD + T] = aw
    qT_np = np.ascontiguousarray(query.T.astype(np.float16))                    # (1024, 256)
    pmT_np = np.ascontiguousarray(
        pm_np.transpose(0, 2, 1).astype(np.float16))  # (B, 128, 1024)
    wqT_np = np.ascontiguousarray(w_query.T.astype(np.float16))                 # (1024, 128)
    w2_np = np.ascontiguousarray(
        np.einsum("af,fck->cka", w_loc_lin, w_loc_conv)
        .reshape(CK, ATT_DIM).astype(np.float16))
    vcol_np = np.ascontiguousarray(v.reshape(ATT_DIM, 1).astype(np.float16))
    ident_np = np.eye(128, dtype=np.float32)
    mneg_np = np.where(mask, np.float32(-1e30), np.float32(0)).astype(np.float32)

    in_maps = []
    for i in range(NCORES):
        s = slice(i * BL, (i + 1) * BL)
        in_maps.append({
            "awp": np.ascontiguousarray(awp[s]),
            "qT": np.ascontiguousarray(qT_np[:, s]),
            "pmT": pmT_np[s],
            "mem": memory[s],
            "mneg": np.ascontiguousarray(mneg_np[s]),
            "wqT": wqT_np,
            "w2": w2_np,
            "vcol": vcol_np,
            "ident": ident_np,
        })
    return in_maps


def kernel(query, memory, processed_memory, attention_weights_cat, mask,
           w_query, w_loc_conv, w_loc_lin, v):
    in_maps = host_prep(query, memory, processed_memory, attention_weights_cat,
                        mask, w_query, w_loc_conv, w_loc_lin, v)
    nc = get_nc()
    res = run_bass_kernel_spmd(nc, in_maps, list(range(NCORES)))
    ctx = np.concatenate([r["ctx_out"] for r in res.results], axis=0)
    attw = np.concatenate([r["attw_out"] for r in res.results], axis=0)
    return ctx, attw


# revision 27
# speedup vs baseline: 1.0039x; 1.0039x over previous
"""Trainium2 Bass kernel for location-sensitive attention (Tacotron-style).

Computes, for B=256, T=1024, RNN_DIM=1024, EMB_DIM=512, ATT_DIM=128:
  pq   = query @ w_query.T                      (B, 128)
  conv = conv1d(attention_weights_cat, w)       (B, 32, T)  'same' pad
  ploc = conv.T @ w_loc_lin.T                   (B, T, 128)
  e    = tanh(pq + ploc + processed_memory) @ v (B, T)
  w    = softmax(e + mask_bias, axis=T)         (B, T)
  ctx  = w @ memory                             (B, 512)
returns (ctx, w).

Sharding: pure data parallel over batch, 32 rows per core on 8 cores.

Design notes (measured on trn2, NEFF exec ~414us vs ~675us for the first
correct fp32 version):
  - fp32 matmuls on trn2 run 2-pass (LOW_HIGH) at ~1/4 bf16 streaming rate,
    so every bandwidth-relevant matmul runs in fp16 with f32 PSUM
    accumulation (end-to-end rel err ~3.5e-4; set CTX_DT = F32 to trade
    ~180us for ~1e-6 accuracy).
  - conv+linear are folded on the host into one (62, 128) matrix W2; the
    conv becomes 2 matmuls per row over an im2col built by
    overlapping-window DMA reads of the host-padded attention weights,
    batched 4 rows per DMA.  Output lands in [a, t] layout.
  - processed_memory is transposed to [a, t] and cast to fp16 on the host;
    VectorE adds it to the conv PSUM; ScalarE applies tanh with the pq
    per-partition bias fused in.
  - energies = v^T @ tanh(...) (M=1 matvecs) are staged through partition 0
    and spread into per-half [16, T] softmax tiles by SBUF->SBUF DMAs;
    softmax is batched across partitions; weights are PE-transposed into
    [t, row] fp16 columns for the context step.
  - context: per row, 8 accumulating [128_t,1]x[128_t,512] fp16 matvecs
    over memory tiles cast f32->fp16 during the (SWDGE) DMA; the second
    half's energies overlap the first half's context to keep HBM busy.
  - phase-1 work is software-pipelined in groups (front: conv/add/tanh;
    back: v-reduce/stage) to hide cross-engine semaphore latency.
"""
import numpy as np

import concourse.bass as bass
import concourse.bacc as bacc
import concourse.mybir as mybir
import concourse.tile as tile
from concourse.bass_utils import run_bass_kernel_spmd

F32 = mybir.dt.float32
# context-path dtype: fp16 keeps ctx relerr ~3e-4 (bf16: 2.5e-3, f32 exact
# but 2x matmul passes).
CTX_DT = mybir.dt.float16

B, T = 256, 1024
RNN_DIM, EMB_DIM, ATT_DIM = 1024, 512, 128
N_FILT, KSIZE = 32, 31
PAD = (KSIZE - 1) // 2
NCORES = 8
BL = B // NCORES          # 32 batch rows per core
CK = 2 * KSIZE            # 62 im2col rows
PADT = T + 2 * PAD        # 1054
NTCH = T // 128           # 8 t-chunks of 128


def build_nc():
    nc = bacc.Bacc("TRN2", target_bir_lowering=False)

    # ---- per-core inputs ----
    awp = nc.declare_dram_parameter("awp", [BL, 2, PADT], CTX_DT, isOutput=False)
    qT = nc.declare_dram_parameter("qT", [RNN_DIM, BL], F32, isOutput=False)
    pmT = nc.declare_dram_parameter("pmT", [BL, ATT_DIM, T], CTX_DT, isOutput=False)
    mem = nc.declare_dram_parameter("mem", [BL, T, EMB_DIM], F32, isOutput=False)
    mneg = nc.declare_dram_parameter("mneg", [BL, T], F32, isOutput=False)
    # ---- shared params ----
    wqT = nc.declare_dram_parameter("wqT", [RNN_DIM, ATT_DIM], F32, isOutput=False)
    w2 = nc.declare_dram_parameter("w2", [CK, ATT_DIM], CTX_DT, isOutput=False)
    vcol = nc.declare_dram_parameter("vcol", [ATT_DIM, 1], CTX_DT, isOutput=False)
    ident = nc.declare_dram_parameter("ident", [128, 128], F32, isOutput=False)
    # ---- outputs ----
    ctx_out = nc.declare_dram_parameter("ctx_out", [BL, EMB_DIM], F32, isOutput=True)
    attw_out = nc.declare_dram_parameter("attw_out", [BL, T], F32, isOutput=True)

    with tile.TileContext(nc) as tc:
        with (
            tc.tile_pool(name="const", bufs=1) as cp,
            tc.tile_pool(name="xp", bufs=4) as xp,
            tc.tile_pool(name="pmp", bufs=4) as pmp,
            tc.tile_pool(name="sp", bufs=4) as sp,
            tc.tile_pool(name="memp", bufs=10) as memp,
            tc.tile_pool(name="stg", bufs=2) as stg,
            tc.tile_pool(name="pta", bufs=2, space="PSUM") as pta,
            tc.tile_pool(name="psB", bufs=4, space="PSUM") as psB,
        ):
            # ---------- constants ----------
            wqT_t = cp.tile([128, RNN_DIM // 128, ATT_DIM], F32, tag="wqT")
            nc.sync.dma_start(
                wqT_t[:], wqT[:].rearrange("(ih il) a -> il ih a", il=128))
            qT_t = cp.tile([128, RNN_DIM // 128, BL], F32, tag="qT")
            nc.sync.dma_start(
                qT_t[:], qT[:].rearrange("(ih il) b -> il ih b", il=128))
            w2_t = cp.tile([CK, ATT_DIM], CTX_DT, tag="w2")
            nc.sync.dma_start(w2_t[:], w2[:])
            v_t = cp.tile([ATT_DIM, 1], CTX_DT, tag="v")
            nc.sync.dma_start(v_t[:], vcol[:])
            id_t = cp.tile([128, 128], F32, tag="id")
            nc.sync.dma_start(id_t[:], ident[:])
            mneg_half = []
            for x in range(2):
                mh = cp.tile([BL // 2, T], F32, tag=f"mneg{x}", name=f"mneg{x}")
                nc.sync.dma_start(mh[:], mneg[x * (BL // 2):(x + 1) * (BL // 2)])
                mneg_half.append(mh)

            pq_sb = cp.tile([ATT_DIM, BL], F32, tag="pq")
            HB = BL // 2  # half-batch: softmax/context run per 16-row half
            e_half = [cp.tile([HB, T], F32, tag=f"e{x}", name=f"e_sb{x}")
                      for x in range(2)]
            w_half = [cp.tile([HB, T], F32, tag=f"w{x}", name=f"w_sb{x}")
                      for x in range(2)]
            wT_half = [cp.tile([128, NTCH, HB], CTX_DT, tag=f"wT{x}",
                               name=f"wT_sb{x}") for x in range(2)]
            stat_half = [cp.tile([HB, 4], F32, tag=f"st{x}", name=f"stat{x}")
                         for x in range(2)]

            # ---------- pq = w_query @ query^T : [128_a, BL] ----------
            pqp = pta.tile([ATT_DIM, BL], F32, tag="pta", name="pqp")
            for c in range(RNN_DIM // 128):
                nc.tensor.matmul(
                    pqp[:], wqT_t[:, c, :], qT_t[:, c, :],
                    start=(c == 0), stop=(c == RNN_DIM // 128 - 1))
            nc.vector.tensor_copy(pq_sb[:], pqp[:])

            # ---------- mem streaming (cast to CTX_DT during DMA) ----------
            mem_tiles = {}

            def load_mem(b, h):
                mt = memp.tile([128, 4, 512], CTX_DT, tag="mt", name=f"mt_{b}_{h}")
                nc.gpsimd.dma_start(
                    mt[:],
                    mem[b, h * 512:(h + 1) * 512, :].rearrange(
                        "(th tl) d -> tl th d", tl=128))
                mem_tiles[(b, h)] = mt

            prefetch = [(b, h) for b in range(BL) for h in range(2)][:12]

            # ---------- phase 1: energies ----------
            # Software-pipelined by groups of GRP rows: the PE stream is
            # [convs(g), vreds(g-1)] so cross-engine latency (PE->DVE add ->
            # ACT tanh -> PE vred) is hidden a group at a time instead of
            # stalling every row.
            GRP = 4
            LOOKAHEAD = 12
            QB = 4   # rows per input-DMA batch
            th_tiles = {}
            ev_tiles = {}
            estages = {}
            in_tiles = {}

            def emit_dma(b0):
                # one batched im2col DMA pair + one pm DMA covering QB rows
                if b0 >= BL:
                    return
                xt = xp.tile([CK, QB, T], CTX_DT, tag="x", name=f"x_{b0}")
                for c2 in range(2):
                    nc.sync.dma_start(
                        xt[c2 * KSIZE:(c2 + 1) * KSIZE, :, :],
                        bass.AP(awp, (b0 * 2 + c2) * PADT,
                                [[1, KSIZE], [2 * PADT, QB], [1, T]]))
                pmt = pmp.tile([128, QB, T], CTX_DT, tag="pm", name=f"pm_{b0}")
                nc.sync.dma_start(
                    pmt[:], pmT[b0:b0 + QB].rearrange("b a t -> a b t"))
                for b in range(b0, b0 + QB):
                    in_tiles[b] = (xt, pmt)

            for b0 in range(0, LOOKAHEAD, QB):
                emit_dma(b0)

            def emit_front(b):
                xt4, pmt4 = in_tiles.pop(b)
                if b % QB == 0:
                    emit_dma(b + LOOKAHEAD)
                xt = xt4[:, b % QB, :]
                pmt = pmt4[:, b % QB, :]
                pl = pta.tile([128, T], F32, tag="pta", name=f"pl_{b}")
                for h in range(2):
                    nc.tensor.matmul(
                        pl[:, h * 512:(h + 1) * 512], w2_t[:],
                        xt[:, h * 512:(h + 1) * 512],
                        start=True, stop=True)
                s = sp.tile([128, T], F32, tag="s", bufs=4, name=f"s_{b}")
                nc.vector.tensor_add(s[:], pl[:], pmt[:])
                th = sp.tile([128, T], CTX_DT, tag="ths", bufs=8,
                             name=f"th_{b}")
                nc.scalar.activation(
                    th[:], s[:], mybir.ActivationFunctionType.Tanh,
                    bias=pq_sb[:, b:b + 1], scale=1.0)
                th_tiles[b] = th

            def emit_back(b):
                if b % 2 == 0:
                    estages[b // 2] = stg.tile([1, 2 * T], F32, tag="est",
                                               bufs=3, name=f"est_{b}")
                estage = estages[b // 2]
                th = th_tiles.pop(b)
                for h in range(2):
                    ev = psB.tile([1, 512], F32, tag="vec", name=f"ev_{b}_{h}")
                    nc.tensor.matmul(ev[:], v_t[:],
                                     th[:, h * 512:(h + 1) * 512],
                                     start=True, stop=True)
                    ev_tiles[(b, h)] = (ev, estage)

            def drain_back(b):
                for h in range(2):
                    ev, estage = ev_tiles.pop((b, h))
                    nc.vector.tensor_copy(
                        estage[0:1,
                               (b % 2) * T + h * 512:(b % 2) * T + (h + 1) * 512],
                        ev[:])
                if b % 2 == 1:
                    eh = e_half[b // HB]
                    r = b % HB
                    nc.gpsimd.dma_start(eh[r - 1:r + 1, :],
                                        estages.pop(b // 2)[:])

            def softmax_half(x):
                eh, wh, st, mh = e_half[x], w_half[x], stat_half[x], mneg_half[x]
                nc.vector.tensor_add(eh[:], eh[:], mh[:])
                nc.vector.reduce_max(st[:, 0:1], eh[:], axis=mybir.AxisListType.X)
                nc.vector.tensor_scalar_mul(st[:, 1:2], st[:, 0:1], -1.0)
                nc.scalar.activation(
                    wh[:], eh[:], mybir.ActivationFunctionType.Exp,
                    bias=st[:, 1:2], scale=1.0, accum_out=st[:, 2:3])
                nc.vector.reciprocal(st[:, 3:4], st[:, 2:3])
                nc.vector.tensor_scalar_mul(wh[:], wh[:], st[:, 3:4])
                nc.gpsimd.dma_start(attw_out[x * HB:(x + 1) * HB, :], wh[:])
                for c in range(NTCH):
                    wTp = pta.tile([128, HB], F32, tag="pta", name=f"wTp{x}_{c}")
                    nc.tensor.matmul(
                        wTp[:], wh[:, c * 128:(c + 1) * 128], id_t[0:HB, 0:HB],
                        is_transpose=True, start=True, stop=True)
                    nc.vector.tensor_copy(wT_half[x][:, c, :], wTp[:])

            cstages = {}

            def emit_ctx(b):
                for h in range(2):
                    if (b, h) not in mem_tiles:
                        load_mem(b, h)
                if b % 4 == 0:
                    cstages[b // 4] = stg.tile([1, 4 * EMB_DIM], F32, tag="cst",
                                               name=f"cst_{b}")
                cstage = cstages[b // 4]
                cx = psB.tile([1, EMB_DIM], F32, tag="vec", name=f"cx_{b}")
                wTh = wT_half[b // HB]
                r = b % HB
                for h in range(2):
                    mt = mem_tiles.pop((b, h))
                    for j in range(4):
                        c = h * 4 + j
                        nc.tensor.matmul(
                            cx[:], wTh[:, c, r:r + 1], mt[:, j, :],
                            start=(c == 0), stop=(c == 7))
                nc.scalar.copy(
                    cstage[0:1, (b % 4) * EMB_DIM:(b % 4 + 1) * EMB_DIM], cx[:])
                if b % 4 == 3:
                    nc.gpsimd.dma_start(ctx_out[b - 3:b + 1, :],
                                        cstages.pop(b // 4)[:])

            # half A energies
            for b in range(HB):
                emit_front(b)
                if b - GRP >= 0:
                    emit_back(b - GRP)
                    drain_back(b - GRP)
                if b % 4 == 1 and prefetch:
                    for _ in range(2):
                        if prefetch:
                            load_mem(*prefetch.pop(0))
            for b in range(HB - GRP, HB):
                emit_back(b)
                drain_back(b)
            softmax_half(0)
            # half B energies interleaved with half A context
            for i in range(HB):
                b = HB + i
                emit_front(b)
                if b - GRP >= HB:
                    emit_back(b - GRP)
                    drain_back(b - GRP)
                emit_ctx(i)
            for b in range(BL - GRP, BL):
                emit_back(b)
                drain_back(b)
            softmax_half(1)
            for i in range(HB, BL):
                emit_ctx(i)

    nc.compile()
    return nc


_NC = None


def get_nc():
    global _NC
    if _NC is None:
        _NC = build_nc()
    return _NC


def host_prep(query, memory, processed_memory, attention_weights_cat, mask,
              w_query, w_loc_conv, w_loc_lin, v):
    """Build the per-core input maps (host-side sharding + weight folding)."""
    query = np.asarray(query, np.float32)
    memory = np.ascontiguousarray(np.asarray(memory, np.float32))
    pm_np = np.asarray(processed_memory, np.float32)
    aw = np.asarray(attention_weights_cat, np.float32)
    mask = np.asarray(mask)
    w_query = np.asarray(w_query, np.float32)
    w_loc_conv = np.asarray(w_loc_conv, np.float32)
    w_loc_lin = np.asarray(w_loc_lin, np.float32)
    v = np.asarray(v, np.float32)

    awp = np.zeros((B, 2, PADT), np.float16)
    awp[:, :, PAD:PAD + T] = aw
    qT_np = np.ascontiguousarray(query.T)                    # (1024, 256)
    pmT_np = np.ascontiguousarray(
        pm_np.transpose(0, 2, 1).astype(np.float16))  # (B, 128, 1024)
    wqT_np = np.ascontiguousarray(w_query.T)                 # (1024, 128)
    w2_np = np.ascontiguousarray(
        np.einsum("af,fck->cka", w_loc_lin, w_loc_conv)
        .reshape(CK, ATT_DIM).astype(np.float16))
    vcol_np = np.ascontiguousarray(v.reshape(ATT_DIM, 1).astype(np.float16))
    ident_np = np.eye(128, dtype=np.float32)
    mneg_np = np.where(mask, np.float32(-1e30), np.float32(0)).astype(np.float32)

    in_maps = []
    for i in range(NCORES):
        s = slice(i * BL, (i + 1) * BL)
        in_maps.append({
            "awp": np.ascontiguousarray(awp[s]),
            "qT": np.ascontiguousarray(qT_np[:, s]),
            "pmT": pmT_np[s],
            "mem": memory[s],
            "mneg": np.ascontiguousarray(mneg_np[s]),
            "wqT": wqT_np,
            "w2": w2_np,
            "vcol": vcol_np,
            "ident": ident_np,
        })
    return in_maps


def kernel(query, memory, processed_memory, attention_weights_cat, mask,
           w_query, w_loc_conv, w_loc_lin, v):
    in_maps = host_prep(query, memory, processed_memory, attention_weights_cat,
                        mask, w_query, w_loc_conv, w_loc_lin, v)
    nc = get_nc()
    res = run_bass_kernel_spmd(nc, in_maps, list(range(NCORES)))
    ctx = np.concatenate([r["ctx_out"] for r in res.results], axis=0)
    attw = np.concatenate([r["attw_out"] for r in res.results], axis=0)
    return ctx, attw


# revision 28
# speedup vs baseline: 1.0474x; 1.0433x over previous
"""Trainium2 Bass kernel for location-sensitive attention (Tacotron-style).

Computes, for B=256, T=1024, RNN_DIM=1024, EMB_DIM=512, ATT_DIM=128:
  pq   = query @ w_query.T                      (B, 128)
  conv = conv1d(attention_weights_cat, w)       (B, 32, T)  'same' pad
  ploc = conv.T @ w_loc_lin.T                   (B, T, 128)
  e    = tanh(pq + ploc + processed_memory) @ v (B, T)
  w    = softmax(e + mask_bias, axis=T)         (B, T)
  ctx  = w @ memory                             (B, 512)
returns (ctx, w).

Sharding: pure data parallel over batch, 32 rows per core on 8 cores.

Design notes (measured on trn2, NEFF exec ~414us vs ~675us for the first
correct fp32 version):
  - fp32 matmuls on trn2 run 2-pass (LOW_HIGH) at ~1/4 bf16 streaming rate,
    so every bandwidth-relevant matmul runs in fp16 with f32 PSUM
    accumulation (end-to-end rel err ~3.5e-4; set CTX_DT = F32 to trade
    ~180us for ~1e-6 accuracy).
  - conv+linear are folded on the host into one (62, 128) matrix W2; the
    conv becomes 2 matmuls per row over an im2col built by
    overlapping-window DMA reads of the host-padded attention weights,
    batched 4 rows per DMA.  Output lands in [a, t] layout.
  - processed_memory is transposed to [a, t] and cast to fp16 on the host;
    VectorE adds it to the conv PSUM; ScalarE applies tanh with the pq
    per-partition bias fused in.
  - energies = v^T @ tanh(...) (M=1 matvecs) are staged through partition 0
    and spread into per-half [16, T] softmax tiles by SBUF->SBUF DMAs;
    softmax is batched across partitions; weights are PE-transposed into
    [t, row] fp16 columns for the context step.
  - context: per row, 8 accumulating [128_t,1]x[128_t,512] fp16 matvecs
    over memory tiles cast f32->fp16 during the (SWDGE) DMA; the second
    half's energies overlap the first half's context to keep HBM busy.
  - phase-1 work is software-pipelined in groups (front: conv/add/tanh;
    back: v-reduce/stage) to hide cross-engine semaphore latency.
"""
import numpy as np

import concourse.bass as bass
import concourse.bacc as bacc
import concourse.mybir as mybir
import concourse.tile as tile
from concourse.bass_utils import run_bass_kernel_spmd

F32 = mybir.dt.float32
# context-path dtype: fp16 keeps ctx relerr ~3e-4 (bf16: 2.5e-3, f32 exact
# but 2x matmul passes).
CTX_DT = mybir.dt.float16

B, T = 256, 1024
RNN_DIM, EMB_DIM, ATT_DIM = 1024, 512, 128
N_FILT, KSIZE = 32, 31
PAD = (KSIZE - 1) // 2
NCORES = 8
BL = B // NCORES          # 32 batch rows per core
CK = 2 * KSIZE            # 62 im2col rows
PADT = T + 2 * PAD        # 1054
NTCH = T // 128           # 8 t-chunks of 128


def build_nc():
    nc = bacc.Bacc("TRN2", target_bir_lowering=False)

    # ---- per-core inputs ----
    awp = nc.declare_dram_parameter("awp", [BL, 2, PADT], CTX_DT, isOutput=False)
    qT = nc.declare_dram_parameter("qT", [RNN_DIM, BL], F32, isOutput=False)
    pmT = nc.declare_dram_parameter("pmT", [BL, ATT_DIM, T], CTX_DT, isOutput=False)
    mem = nc.declare_dram_parameter("mem", [BL, T, EMB_DIM], F32, isOutput=False)
    mneg = nc.declare_dram_parameter("mneg", [BL, T], F32, isOutput=False)
    # ---- shared params ----
    wqT = nc.declare_dram_parameter("wqT", [RNN_DIM, ATT_DIM], F32, isOutput=False)
    w2 = nc.declare_dram_parameter("w2", [CK, ATT_DIM], CTX_DT, isOutput=False)
    vcol = nc.declare_dram_parameter("vcol", [ATT_DIM, 1], CTX_DT, isOutput=False)
    ident = nc.declare_dram_parameter("ident", [128, 128], F32, isOutput=False)
    # ---- outputs ----
    ctx_out = nc.declare_dram_parameter("ctx_out", [BL, EMB_DIM], F32, isOutput=True)
    attw_out = nc.declare_dram_parameter("attw_out", [BL, T], F32, isOutput=True)

    with tile.TileContext(nc) as tc:
        with (
            tc.tile_pool(name="const", bufs=1) as cp,
            tc.tile_pool(name="xp", bufs=4) as xp,
            tc.tile_pool(name="pmp", bufs=4) as pmp,
            tc.tile_pool(name="sp", bufs=4) as sp,
            tc.tile_pool(name="memp", bufs=10) as memp,
            tc.tile_pool(name="stg", bufs=2) as stg,
            tc.tile_pool(name="pta", bufs=2, space="PSUM") as pta,
            tc.tile_pool(name="psB", bufs=4, space="PSUM") as psB,
        ):
            # ---------- constants ----------
            wqT_t = cp.tile([128, RNN_DIM // 128, ATT_DIM], F32, tag="wqT")
            nc.sync.dma_start(
                wqT_t[:], wqT[:].rearrange("(ih il) a -> il ih a", il=128))
            qT_t = cp.tile([128, RNN_DIM // 128, BL], F32, tag="qT")
            nc.sync.dma_start(
                qT_t[:], qT[:].rearrange("(ih il) b -> il ih b", il=128))
            w2_t = cp.tile([CK, ATT_DIM], CTX_DT, tag="w2")
            nc.sync.dma_start(w2_t[:], w2[:])
            v_t = cp.tile([ATT_DIM, 1], CTX_DT, tag="v")
            nc.sync.dma_start(v_t[:], vcol[:])
            id_t = cp.tile([128, 128], F32, tag="id")
            nc.sync.dma_start(id_t[:], ident[:])
            mneg_half = []
            for x in range(2):
                mh = cp.tile([BL // 2, T], F32, tag=f"mneg{x}", name=f"mneg{x}")
                nc.sync.dma_start(mh[:], mneg[x * (BL // 2):(x + 1) * (BL // 2)])
                mneg_half.append(mh)

            pq_sb = cp.tile([ATT_DIM, BL], F32, tag="pq")
            gate_sb = cp.tile([1, 16], CTX_DT, tag="gate")
            HB = BL // 2  # half-batch: softmax/context run per 16-row half
            e_half = [cp.tile([HB, T], F32, tag=f"e{x}", name=f"e_sb{x}")
                      for x in range(2)]
            w_half = [cp.tile([HB, T], F32, tag=f"w{x}", name=f"w_sb{x}")
                      for x in range(2)]
            wT_half = [cp.tile([128, NTCH, HB], CTX_DT, tag=f"wT{x}",
                               name=f"wT_sb{x}") for x in range(2)]
            stat_half = [cp.tile([HB, 4], F32, tag=f"st{x}", name=f"stat{x}")
                         for x in range(2)]

            # ---------- pq = w_query @ query^T : [128_a, BL] ----------
            pqp = pta.tile([ATT_DIM, BL], F32, tag="pta", name="pqp")
            for c in range(RNN_DIM // 128):
                nc.tensor.matmul(
                    pqp[:], wqT_t[:, c, :], qT_t[:, c, :],
                    start=(c == 0), stop=(c == RNN_DIM // 128 - 1))
            nc.vector.tensor_copy(pq_sb[:], pqp[:])

            # ---------- mem streaming (cast to CTX_DT during DMA) ----------
            mem_tiles = {}

            def load_mem(b, h):
                mt = memp.tile([128, 4, 512], CTX_DT, tag="mt", name=f"mt_{b}_{h}")
                nc.gpsimd.dma_start(
                    mt[:],
                    mem[b, h * 512:(h + 1) * 512, :].rearrange(
                        "(th tl) d -> tl th d", tl=128))
                mem_tiles[(b, h)] = mt

            prefetch = [(b, h) for b in range(BL) for h in range(2)][:12]

            # ---------- phase 1: energies ----------
            # Software-pipelined by groups of GRP rows: the PE stream is
            # [convs(g), vreds(g-1)] so cross-engine latency (PE->DVE add ->
            # ACT tanh -> PE vred) is hidden a group at a time instead of
            # stalling every row.
            GRP = 4
            LOOKAHEAD = 12
            QB = 4   # rows per input-DMA batch
            th_tiles = {}
            ev_tiles = {}
            estages = {}
            in_tiles = {}

            def emit_dma(b0):
                # one batched im2col DMA pair + one pm DMA covering QB rows
                if b0 >= BL:
                    return
                xt = xp.tile([CK, QB, T], CTX_DT, tag="x", name=f"x_{b0}")
                for c2 in range(2):
                    nc.sync.dma_start(
                        xt[c2 * KSIZE:(c2 + 1) * KSIZE, :, :],
                        bass.AP(awp, (b0 * 2 + c2) * PADT,
                                [[1, KSIZE], [2 * PADT, QB], [1, T]]))
                pmt = pmp.tile([128, QB, T], CTX_DT, tag="pm", name=f"pm_{b0}")
                nc.sync.dma_start(
                    pmt[:], pmT[b0:b0 + QB].rearrange("b a t -> a b t"))
                for b in range(b0, b0 + QB):
                    in_tiles[b] = (xt, pmt)

            for b0 in range(0, LOOKAHEAD, QB):
                emit_dma(b0)

            def emit_front(b):
                xt4, pmt4 = in_tiles.pop(b)
                if b % QB == 0:
                    emit_dma(b + LOOKAHEAD)
                xt = xt4[:, b % QB, :]
                pmt = pmt4[:, b % QB, :]
                pl = pta.tile([128, T], F32, tag="pta", name=f"pl_{b}")
                for h in range(2):
                    nc.tensor.matmul(
                        pl[:, h * 512:(h + 1) * 512], w2_t[:],
                        xt[:, h * 512:(h + 1) * 512],
                        start=True, stop=True)
                s = sp.tile([128, T], F32, tag="s", bufs=4, name=f"s_{b}")
                nc.vector.tensor_add(s[:], pl[:], pmt[:])
                th = sp.tile([128, T], CTX_DT, tag="ths", bufs=8,
                             name=f"th_{b}")
                nc.scalar.activation(
                    th[:], s[:], mybir.ActivationFunctionType.Tanh,
                    bias=pq_sb[:, b:b + 1], scale=1.0)
                th_tiles[b] = th
                if b == 0:
                    # gate: holds back the gpsimd DMA queue (mem prefetch)
                    # until the first row's compute is flowing, so startup
                    # input DMAs get full HBM bandwidth
                    nc.gpsimd.tensor_copy(gate_sb[:], th[0:1, 0:16])

            def emit_back(b):
                if b % 2 == 0:
                    estages[b // 2] = stg.tile([1, 2 * T], F32, tag="est",
                                               bufs=3, name=f"est_{b}")
                estage = estages[b // 2]
                th = th_tiles.pop(b)
                for h in range(2):
                    ev = psB.tile([1, 512], F32, tag="vec", name=f"ev_{b}_{h}")
                    nc.tensor.matmul(ev[:], v_t[:],
                                     th[:, h * 512:(h + 1) * 512],
                                     start=True, stop=True)
                    ev_tiles[(b, h)] = (ev, estage)

            def drain_back(b):
                for h in range(2):
                    ev, estage = ev_tiles.pop((b, h))
                    nc.vector.tensor_copy(
                        estage[0:1,
                               (b % 2) * T + h * 512:(b % 2) * T + (h + 1) * 512],
                        ev[:])
                if b % 2 == 1:
                    eh = e_half[b // HB]
                    r = b % HB
                    nc.gpsimd.dma_start(eh[r - 1:r + 1, :],
                                        estages.pop(b // 2)[:])

            def softmax_half(x):
                eh, wh, st, mh = e_half[x], w_half[x], stat_half[x], mneg_half[x]
                nc.vector.tensor_add(eh[:], eh[:], mh[:])
                nc.vector.reduce_max(st[:, 0:1], eh[:], axis=mybir.AxisListType.X)
                nc.vector.tensor_scalar_mul(st[:, 1:2], st[:, 0:1], -1.0)
                nc.scalar.activation(
                    wh[:], eh[:], mybir.ActivationFunctionType.Exp,
                    bias=st[:, 1:2], scale=1.0, accum_out=st[:, 2:3])
                nc.vector.reciprocal(st[:, 3:4], st[:, 2:3])
                nc.vector.tensor_scalar_mul(wh[:], wh[:], st[:, 3:4])
                nc.gpsimd.dma_start(attw_out[x * HB:(x + 1) * HB, :], wh[:])
                for c in range(NTCH):
                    wTp = pta.tile([128, HB], F32, tag="pta", name=f"wTp{x}_{c}")
                    nc.tensor.matmul(
                        wTp[:], wh[:, c * 128:(c + 1) * 128], id_t[0:HB, 0:HB],
                        is_transpose=True, start=True, stop=True)
                    nc.vector.tensor_copy(wT_half[x][:, c, :], wTp[:])

            cstages = {}

            def emit_ctx(b):
                for h in range(2):
                    if (b, h) not in mem_tiles:
                        load_mem(b, h)
                if b % 4 == 0:
                    cstages[b // 4] = stg.tile([1, 4 * EMB_DIM], F32, tag="cst",
                                               name=f"cst_{b}")
                cstage = cstages[b // 4]
                cx = psB.tile([1, EMB_DIM], F32, tag="vec", name=f"cx_{b}")
                wTh = wT_half[b // HB]
                r = b % HB
                for h in range(2):
                    mt = mem_tiles.pop((b, h))
                    for j in range(4):
                        c = h * 4 + j
                        nc.tensor.matmul(
                            cx[:], wTh[:, c, r:r + 1], mt[:, j, :],
                            start=(c == 0), stop=(c == 7))
                nc.scalar.copy(
                    cstage[0:1, (b % 4) * EMB_DIM:(b % 4 + 1) * EMB_DIM], cx[:])
                if b % 4 == 3:
                    nc.gpsimd.dma_start(ctx_out[b - 3:b + 1, :],
                                        cstages.pop(b // 4)[:])

            # half A energies
            for b in range(HB):
                emit_front(b)
                if b - GRP >= 0:
                    emit_back(b - GRP)
                    drain_back(b - GRP)
                if b % 4 == 1 and prefetch:
                    for _ in range(2):
                        if prefetch:
                            load_mem(*prefetch.pop(0))
            for b in range(HB - GRP, HB):
                emit_back(b)
                drain_back(b)
            softmax_half(0)
            # half B energies interleaved with half A context
            for i in range(HB):
                b = HB + i
                emit_front(b)
                if b - GRP >= HB:
                    emit_back(b - GRP)
                    drain_back(b - GRP)
                emit_ctx(i)
            for b in range(BL - GRP, BL):
                emit_back(b)
                drain_back(b)
            softmax_half(1)
            for i in range(HB, BL):
                emit_ctx(i)

    nc.compile()
    return nc


_NC = None


def get_nc():
    global _NC
    if _NC is None:
        _NC = build_nc()
    return _NC


def host_prep(query, memory, processed_memory, attention_weights_cat, mask,
              w_query, w_loc_conv, w_loc_lin, v):
    """Build the per-core input maps (host-side sharding + weight folding)."""
    query = np.asarray(query, np.float32)
    memory = np.ascontiguousarray(np.asarray(memory, np.float32))
    pm_np = np.asarray(processed_memory, np.float32)
    aw = np.asarray(attention_weights_cat, np.float32)
    mask = np.asarray(mask)
    w_query = np.asarray(w_query, np.float32)
    w_loc_conv = np.asarray(w_loc_conv, np.float32)
    w_loc_lin = np.asarray(w_loc_lin, np.float32)
    v = np.asarray(v, np.float32)

    awp = np.zeros((B, 2, PADT), np.float16)
    awp[:, :, PAD:PAD + T] = aw
    qT_np = np.ascontiguousarray(query.T)                    # (1024, 256)
    pmT_np = np.ascontiguousarray(
        pm_np.transpose(0, 2, 1).astype(np.float16))  # (B, 128, 1024)
    wqT_np = np.ascontiguousarray(w_query.T)                 # (1024, 128)
    w2_np = np.ascontiguousarray(
        np.einsum("af,fck->cka", w_loc_lin, w_loc_conv)
        .reshape(CK, ATT_DIM).astype(np.float16))
    vcol_np = np.ascontiguousarray(v.reshape(ATT_DIM, 1).astype(np.float16))
    ident_np = np.eye(128, dtype=np.float32)
    mneg_np = np.where(mask, np.float32(-1e30), np.float32(0)).astype(np.float32)

    in_maps = []
    for i in range(NCORES):
        s = slice(i * BL, (i + 1) * BL)
        in_maps.append({
            "awp": np.ascontiguousarray(awp[s]),
            "qT": np.ascontiguousarray(qT_np[:, s]),
            "pmT": pmT_np[s],
            "mem": memory[s],
            "mneg": np.ascontiguousarray(mneg_np[s]),
            "wqT": wqT_np,
            "w2": w2_np,
            "vcol": vcol_np,
            "ident": ident_np,
        })
    return in_maps


def kernel(query, memory, processed_memory, attention_weights_cat, mask,
           w_query, w_loc_conv, w_loc_lin, v):
    in_maps = host_prep(query, memory, processed_memory, attention_weights_cat,
                        mask, w_query, w_loc_conv, w_loc_lin, v)
    nc = get_nc()
    res = run_bass_kernel_spmd(nc, in_maps, list(range(NCORES)))
    ctx = np.concatenate([r["ctx_out"] for r in res.results], axis=0)
    attw = np.concatenate([r["attw_out"] for r in res.results], axis=0)
    return ctx, attw
